# revision 1
# baseline (speedup 1.0000x reference)
"""Trainium2 Bass kernel for nn_CNNPredictor (attention scorer + CNN head).

Sharding: data-parallel over batch b (8 batches -> 8 NeuronCores), no
collectives. Each core computes its batch's [TYPE_NUM] output row; host
gathers to [B, TYPE_NUM].

Math (per batch):
  pre[c,t,:] = [q|ctx|, |q-ctx|, q*ctx] @ W_h.T + b_h   (4e = 1024 hidden)
split as
  pre = A[c] + B[t] + W3 @ |q-ctx| + W4 @ (q*ctx)
with A = q @ W1.T (tiny), B = ctx @ W2.T + b_h (tiny). A/B are folded into
the PSUM accumulation with constant 0/1 indicator matmuls, so the big
contraction is K=512 instead of K=1024. Only t-positions with mask==1 are
computed (padded to a multiple of 8); masked softmax handles the padding.
"""

import os
import sys

for _p in ("/opt/trn_rl_repo",):
    if _p not in sys.path:
        sys.path.append(_p)

import numpy as np
from ml_dtypes import bfloat16

import concourse.bass as bass
import concourse.bacc as bacc
import concourse.tile as tile
from concourse import mybir
from concourse.bass_utils import run_bass_kernel_spmd
from concourse.bass_interp import get_hw_module

F32 = mybir.dt.float32
BF16 = mybir.dt.bfloat16
AF = mybir.ActivationFunctionType
ALU = mybir.AluOpType

B, C, T, E = 8, 64, 128, 256
H = 4 * E  # 1024
NF, TYPE_NUM = 128, 40
KS = (5, 4, 3)
NEG = -1e10
NUM_CORES = 8

# module-level knobs for test harness
TRACE = False
LAST_EXEC_NS = None

_CACHE = {}


def _build_program(n_pad):
    """Build the SPMD Bass program for padded active length n_pad (mult of 8)."""
    stage = int(os.environ.get("KSTAGE", "99"))
    R = n_pad // 8  # number of 512-wide r tiles; r = (t, c) t-major

    nc = bacc.Bacc("TRN2", target_bir_lowering=False, debug=False,
                   num_devices=NUM_CORES)

    d_WhT = nc.dram_tensor("WhT", [128, 8, H], BF16, kind="ExternalInput")
    d_qT = nc.dram_tensor("qT", [128, 2, C], BF16, kind="ExternalInput")
    d_ctxT = nc.dram_tensor("ctxT", [128, 2, n_pad], BF16, kind="ExternalInput")
    d_ctx = nc.dram_tensor("ctx", [n_pad, E], BF16, kind="ExternalInput")
    d_Wv = nc.dram_tensor("Wv", [128, 8], BF16, kind="ExternalInput")
    d_bh = nc.dram_tensor("bh", [1, H], BF16, kind="ExternalInput")
    d_maskadd = nc.dram_tensor("maskadd", [C, n_pad], F32, kind="ExternalInput")
    d_IndA = nc.dram_tensor("IndA", [C, 512], BF16, kind="ExternalInput")
    d_IndB = nc.dram_tensor("IndB", [n_pad, n_pad * C], BF16, kind="ExternalInput")
    d_WlT = nc.dram_tensor("WlT", [128, 8, E], BF16, kind="ExternalInput")
    d_bl = nc.dram_tensor("bl", [128, 2], F32, kind="ExternalInput")
    d_cw = [nc.dram_tensor(f"cw{i}", [128, KS[i], 2, NF], BF16,
                           kind="ExternalInput") for i in range(3)]
    d_cb = nc.dram_tensor("cb", [1, 3 * NF], BF16, kind="ExternalInput")
    d_WcT = nc.dram_tensor("WcT", [128, 3, TYPE_NUM], BF16, kind="ExternalInput")
    d_bc = nc.dram_tensor("bc", [TYPE_NUM, 1], F32, kind="ExternalInput")
    d_out = nc.dram_tensor("out", [TYPE_NUM], F32, kind="ExternalOutput")

    with tile.TileContext(nc) as tc:
        with (
            tc.tile_pool(name="const", bufs=1) as cpool,
            tc.tile_pool(name="ft", bufs=2) as ftpool,
            tc.tile_pool(name="th", bufs=9) as thpool,
            tc.tile_pool(name="soft", bufs=1) as spool,
            tc.tile_pool(name="ps_main", bufs=3, space="PSUM") as ps_main,
            tc.tile_pool(name="ps_sc", bufs=2, space="PSUM") as ps_sc,
            tc.tile_pool(name="ps_sm", bufs=2, space="PSUM") as ps_sm,
            tc.tile_pool(name="drp", bufs=1, space="DRAM") as drpool,
        ):
            d_scr = drpool.tile([n_pad, C], F32)
            # ---- load constants -------------------------------------------
            WhT = cpool.tile([128, 8, H], BF16)
            for kc in range(8):
                nc.sync.dma_start(out=WhT[:, kc, :], in_=d_WhT[:, kc, :])
            qT = cpool.tile([128, 2, C], BF16)
            nc.sync.dma_start(out=qT[:], in_=d_qT[:])
            ctxT = cpool.tile([128, 2, n_pad], BF16)
            nc.sync.dma_start(out=ctxT[:], in_=d_ctxT[:])
            ctxa = cpool.tile([n_pad, E], BF16)
            nc.sync.dma_start(out=ctxa[:], in_=d_ctx[:])
            Wv = cpool.tile([128, 8], BF16)
            nc.sync.dma_start(out=Wv[:], in_=d_Wv[:])
            bh = cpool.tile([1, H], BF16)
            nc.sync.dma_start(out=bh[:], in_=d_bh[:])
            maskadd = cpool.tile([C, n_pad], F32)
            nc.sync.dma_start(out=maskadd[:], in_=d_maskadd[:])
            IndA = cpool.tile([C, 512], BF16)
            nc.sync.dma_start(out=IndA[:], in_=d_IndA[:])
            IndB = cpool.tile([n_pad, n_pad * C], BF16)
            nc.sync.dma_start(out=IndB[:], in_=d_IndB[:])
            WlT = cpool.tile([128, 8, E], BF16)
            nc.sync.dma_start(out=WlT[:], in_=d_WlT[:])
            bl = cpool.tile([128, 2], F32)
            nc.sync.dma_start(out=bl[:], in_=d_bl[:])
            cw = []
            for i in range(3):
                cwt = cpool.tile([128, KS[i], 2, NF], BF16, tag=f"cw{i}")
                nc.sync.dma_start(out=cwt[:], in_=d_cw[i][:])
                cw.append(cwt)
            cb = cpool.tile([1, 3 * NF], BF16)
            nc.sync.dma_start(out=cb[:], in_=d_cb[:])
            WcT = cpool.tile([128, 3, TYPE_NUM], BF16)
            nc.sync.dma_start(out=WcT[:], in_=d_WcT[:])
            bc = cpool.tile([TYPE_NUM, 1], F32)
            nc.sync.dma_start(out=bc[:], in_=d_bc[:])

            ones = cpool.tile([1, max(n_pad, C)], BF16)
            nc.vector.memset(ones[:], 1.0)

            # dense broadcast materializations (step-0 read APs mis-execute
            # on HW DVE): qbc[p, ec, t, c] = qT[p, ec, c]; ctxbc[p, ec, t, c]
            # = ctxT[p, ec, t] -- built by doubling copies.
            qbc = cpool.tile([128, 2, 8, C], BF16)
            nc.vector.tensor_copy(qbc[:, :, 0, :], qT[:])
            nc.vector.tensor_copy(qbc[:, :, 1, :], qbc[:, :, 0, :])
            nc.vector.tensor_copy(qbc[:, :, 2:4, :], qbc[:, :, 0:2, :])
            nc.vector.tensor_copy(qbc[:, :, 4:8, :], qbc[:, :, 0:4, :])
            ctxbc = cpool.tile([128, 2, n_pad, C], BF16)
            nc.vector.tensor_copy(ctxbc[:, :, :, 0], ctxT[:])
            w = 1
            while w < C:
                nc.vector.tensor_copy(ctxbc[:, :, :, w:2 * w],
                                      ctxbc[:, :, :, 0:w])
                w *= 2

            # ---- phase 0: A_T = q @ W1.T ; B_T = ctx @ W2.T + b_h ---------
            A_T = cpool.tile([C, H], BF16)
            B_T = cpool.tile([n_pad, H], BF16)
            for jn in range(2):
                jsl = slice(jn * 512, (jn + 1) * 512)
                psA = ps_sm.tile([C, 512], F32, tag="sm")
                nc.tensor.matmul(psA[:], qT[:, 0, :], WhT[:, 0, jsl],
                                 start=True, stop=False)
                nc.tensor.matmul(psA[:], qT[:, 1, :], WhT[:, 1, jsl],
                                 start=False, stop=True)
                nc.scalar.copy(A_T[:, jsl], psA[:])
                psB = ps_sm.tile([n_pad, 512], F32, tag="sm")
                nc.tensor.matmul(psB[:], ctxT[:, 0, :], WhT[:, 2, jsl],
                                 start=True, stop=False)
                nc.tensor.matmul(psB[:], ctxT[:, 1, :], WhT[:, 3, jsl],
                                 start=False, stop=False)
                nc.tensor.matmul(psB[:], ones[:, :n_pad], bh[:, jsl],
                                 start=False, stop=True)
                nc.scalar.copy(B_T[:, jsl], psB[:])

            if stage < 2:
                nc.gpsimd.dma_start(out=d_out[:], in_=A_T[0:TYPE_NUM, 0])

            # ---- phase 1: scores over (c, active t) -----------------------
            scoresT = spool.tile([C, n_pad], F32)
            if stage >= 2:
                for rt in range(R):
                    ftC = ftpool.tile([128, 2, 8, C], BF16, tag="ftC")
                    ftD = ftpool.tile([128, 2, 8, C], BF16, tag="ftD")
                    for ec in range(2):
                        bq = qbc[:, ec]
                        bcx = ctxbc[:, ec, rt * 8:(rt + 1) * 8, :]
                        nc.vector.tensor_sub(ftC[:, ec], bq, bcx)
                        nc.vector.scalar_tensor_tensor(
                            ftC[:, ec], ftC[:, ec], -1.0, ftC[:, ec],
                            op0=ALU.mult, op1=ALU.max)
                        nc.vector.tensor_mul(ftD[:, ec], bq, bcx)
                    if os.environ.get("KDUMP", "") == "ftd" and rt == 5:
                        nc.gpsimd.dma_start(out=d_out[:],
                                            in_=ftD[0:1, 0, 4, 0:TYPE_NUM])
                    S = ps_sc.tile([1, 512], F32, tag="S")
                    ths = []
                    for jc in range(8):
                        jsl = slice(jc * 128, (jc + 1) * 128)
                        P = ps_main.tile([128, 512], F32, tag="P")
                        nc.tensor.matmul(P[:], WhT[:, 4, jsl],
                                         ftC[:, 0].rearrange("p a b -> p (a b)"),
                                         start=True, stop=False)
                        nc.tensor.matmul(P[:], WhT[:, 5, jsl],
                                         ftC[:, 1].rearrange("p a b -> p (a b)"),
                                         start=False, stop=False)
                        nc.tensor.matmul(P[:], WhT[:, 6, jsl],
                                         ftD[:, 0].rearrange("p a b -> p (a b)"),
                                         start=False, stop=False)
                        nc.tensor.matmul(P[:], WhT[:, 7, jsl],
                                         ftD[:, 1].rearrange("p a b -> p (a b)"),
                                         start=False, stop=True)
                        nc.tensor.matmul(P[:], A_T[:, jsl], IndA[:],
                                         start=False, stop=False,
                                         skip_group_check=True)
                        nc.tensor.matmul(P[:], B_T[:, jsl],
                                         IndB[:, rt * 512:(rt + 1) * 512],
                                         start=False, stop=False,
                                         skip_group_check=True)
                        TH = thpool.tile([128, 512], BF16, tag="TH")
                        nc.scalar.activation(TH[:], P[:], AF.Tanh)
                        ths.append(TH)
                    for jc in range(8):
                        nc.tensor.matmul(S[:], Wv[:, jc:jc + 1], ths[jc][:],
                                         start=(jc == 0), stop=(jc == 7),
                                         skip_group_check=True)
                    S_sb = thpool.tile([1, 512], F32, tag="S_sb")
                    nc.vector.tensor_copy(S_sb[:], S[:])
                    nc.sync.dma_start(
                        out=d_scr[rt * 8:(rt + 1) * 8, :].unsqueeze(0),
                        in_=S_sb[0:1, :].rearrange("p (a b) -> p a b", b=C))
                # gather scr[t*64+c] -> scoresT[c, t]
                nc.sync.dma_start(out=scoresT[:],
                                  in_=d_scr[:].rearrange("t c -> c t"))
            if stage == 2:
                nc.sync.dma_start(out=d_out[:], in_=scoresT[0:TYPE_NUM, 0])

            # ---- masked softmax + g = attn @ ctx --------------------------
            if stage >= 3:
                nc.vector.tensor_add(scoresT[:], scoresT[:], maskadd[:])
                mx = spool.tile([C, 1], F32)
                mxp = spool.tile([C, 1], F32)
                nc.vector.tensor_reduce(mxp[:], scoresT[:],
                                        axis=mybir.AxisListType.X, op=ALU.max)
                nc.vector.tensor_scalar_mul(mx[:], mxp[:], -1.0)  # mx = -max
                ex = spool.tile([C, n_pad], F32)
                se = spool.tile([C, 1], F32)
                nc.scalar.activation(ex[:], scoresT[:], AF.Exp, bias=mx[:],
                                     scale=1.0, accum_out=se[:])
                rse = spool.tile([C, 1], F32)
                nc.vector.reciprocal(rse[:], se[:])
                attn = spool.tile([C, n_pad], BF16)
                nc.vector.tensor_scalar_mul(attn[:], ex[:], rse[:])

                attnT_ps = ps_sm.tile([n_pad, C], BF16, tag="sm")
                nc.tensor.transpose(attnT_ps[:], attn[:], IndA[:, :C])
                attnT = spool.tile([n_pad, C], BF16)
                nc.vector.tensor_copy(attnT[:], attnT_ps[:])
                g_ps = ps_sm.tile([C, E], F32, tag="sm")
                nc.tensor.matmul(g_ps[:], attnT[:], ctxa[:], start=True,
                                 stop=True)
                g_sb = spool.tile([C, E], BF16)
                nc.scalar.copy(g_sb[:], g_ps[:])
                gT = spool.tile([128, 2, C], BF16)
                for ec in range(2):
                    gT_ps = ps_sm.tile([128, C], BF16, tag="sm")
                    nc.tensor.transpose(gT_ps[:],
                                        g_sb[:, ec * 128:(ec + 1) * 128],
                                        IndA[:, :C])
                    nc.vector.tensor_copy(gT[:, ec, :], gT_ps[:])
            if stage == 3:
                dump = os.environ.get("KDUMP", "g")
                if dump == "ctxbc":
                    nc.gpsimd.dma_start(out=d_out[:],
                                        in_=ctxbc[0:1, 0, 44, 0:TYPE_NUM])
                if dump == "bt":
                    nc.gpsimd.dma_start(out=d_out[:],
                                        in_=B_T[44:45, 0:TYPE_NUM])
                dmap = {"g": g_sb[0:TYPE_NUM, 0], "mx": mx[0:TYPE_NUM, 0],
                        "ex": ex[0:TYPE_NUM, 0], "se": se[0:TYPE_NUM, 0],
                        "attn": attn[0:TYPE_NUM, 0],
                        "attnT": attnT[0:TYPE_NUM, 0],
                        "sc": scoresT[0:TYPE_NUM, 0],
                        "sc50": scoresT[0:TYPE_NUM, 50],
                        "sc8": scoresT[0:TYPE_NUM, 8],
                        "row0": scoresT[0, 0:TYPE_NUM],
                        "dscr0": d_scr[0, 0:TYPE_NUM],
                        "dscr50": d_scr[50, 0:TYPE_NUM],
                        "dscr16": d_scr[16, 0:TYPE_NUM],
                        "dscr32": d_scr[32, 0:TYPE_NUM],
                        "dscr40": d_scr[40, 0:TYPE_NUM],
                        "dscr44": d_scr[44, 0:TYPE_NUM],
                        "dscr48": d_scr[48, 0:TYPE_NUM],
                        "dscr56": d_scr[56, 0:TYPE_NUM],
                        "row40": scoresT[0, 32:32 + TYPE_NUM]}
                if dump in dmap:
                    nc.gpsimd.dma_start(out=d_out[:], in_=dmap[dump])

            # ---- phase 2: h2 = tanh([q|g|,|q-g|,q*g] @ Wh.T + bh) ---------
            if stage >= 4:
                f2C = spool.tile([128, 2, C], BF16)
                f2D = spool.tile([128, 2, C], BF16)
                for ec in range(2):
                    nc.vector.tensor_sub(f2C[:, ec], qT[:, ec, :], gT[:, ec, :])
                    nc.vector.scalar_tensor_tensor(
                        f2C[:, ec], f2C[:, ec], -1.0, f2C[:, ec],
                        op0=ALU.mult, op1=ALU.max)
                    nc.vector.tensor_mul(f2D[:, ec], qT[:, ec, :], gT[:, ec, :])
                h2T = spool.tile([128, 8, C], BF16)
                for jc in range(8):
                    jsl = slice(jc * 128, (jc + 1) * 128)
                    H2 = ps_sm.tile([128, C], F32, tag="sm")
                    for mi, rhs_t in enumerate((qT[:, 0, :], qT[:, 1, :],
                                                gT[:, 0, :], gT[:, 1, :],
                                                f2C[:, 0, :], f2C[:, 1, :],
                                                f2D[:, 0, :], f2D[:, 1, :])):
                        nc.tensor.matmul(H2[:], WhT[:, mi, jsl], rhs_t,
                                         start=(mi == 0), stop=False)
                    nc.tensor.matmul(H2[:], bh[:, jsl], ones[:, :C],
                                     start=False, stop=True)
                    nc.scalar.activation(h2T[:, jc, :], H2[:], AF.Tanh)

                # x.T = W_lin @ h2 : [e, c], e-major for the convs
                xT = spool.tile([128, 2, C], BF16)
                for ec2 in range(2):
                    X = ps_sm.tile([128, C], F32, tag="sm")
                    for jc in range(8):
                        nc.tensor.matmul(
                            X[:], WlT[:, jc, ec2 * 128:(ec2 + 1) * 128],
                            h2T[:, jc, :], start=(jc == 0), stop=(jc == 7))
                    nc.scalar.activation(xT[:, ec2, :], X[:], AF.Identity,
                                         bias=bl[:, ec2:ec2 + 1], scale=1.0)

                # convs + relu + maxpool; pooled[f, i]
                pooled_raw = spool.tile([NF, 3], F32)
                for i in range(3):
                    ki = KS[i]
                    oi = C - ki + 1
                    Y = ps_sm.tile([NF, oi], F32, tag="sm")
                    first = True
                    for dk in range(ki):
                        for ec2 in range(2):
                            nc.tensor.matmul(Y[:], cw[i][:, dk, ec2, :],
                                             xT[:, ec2, dk:dk + oi],
                                             start=first, stop=False)
                            first = False
                    nc.tensor.matmul(Y[:], cb[:, i * NF:(i + 1) * NF],
                                     ones[:, :oi], start=False, stop=True)
                    nc.vector.tensor_reduce(pooled_raw[:, i:i + 1], Y[:],
                                            axis=mybir.AxisListType.X,
                                            op=ALU.max)
                pooled = spool.tile([NF, 3], BF16)
                nc.scalar.activation(pooled[:], pooled_raw[:], AF.Relu)

                # final linear: out = W_cnn @ cnn + b_cnn
                O = ps_sm.tile([TYPE_NUM, 1], F32, tag="sm")
                for i in range(3):
                    nc.tensor.matmul(O[:], WcT[:, i, :], pooled[:, i:i + 1],
                                     start=(i == 0), stop=(i == 2))
                out_sb = spool.tile([TYPE_NUM, 1], F32)
                nc.scalar.activation(out_sb[:], O[:], AF.Identity, bias=bc[:],
                                     scale=1.0)
                nc.sync.dma_start(out=d_out[:], in_=out_sb[:, 0])

    nc.compile()
    nc.m = get_hw_module(nc.m)
    return nc


def _prep_inputs(query, context, mask, W_hidden, b_hidden, W_v, b_v,
                 W_lin, b_lin, conv_w0, conv_b0, conv_w1, conv_b1,
                 conv_w2, conv_b2, W_cnn, b_cnn):
    """Host-side layout prep. Returns (n_pad, per_core_maps)."""
    f32 = np.float32
    mask = np.asarray(mask)
    n_act = mask.sum(1)
    if n_act.min() == 0:
        # degenerate: keep every position, mask on device via maskadd
        idxs = [np.arange(T) for _ in range(B)]
        n_pad = T
        mads = [np.where(mask[b] < 1, NEG, 0.0).astype(f32) for b in range(B)]
    else:
        n_pad = max(8, int(-(-int(n_act.max()) // 8) * 8))
        idxs, mads = [], []
        for b in range(B):
            idx = np.nonzero(mask[b])[0]
            ma = np.full(n_pad, NEG, f32)
            ma[:len(idx)] = 0.0
            idx = np.concatenate([idx, np.zeros(n_pad - len(idx), np.int64)])
            idxs.append(idx)
            mads.append(ma)

    bf = bfloat16
    Wh = np.asarray(W_hidden, f32)
    WhT = np.ascontiguousarray(Wh.T).reshape(8, 128, H).transpose(1, 0, 2)
    shared = {
        "WhT": np.ascontiguousarray(WhT).astype(bf),
        "qT": np.ascontiguousarray(
            np.asarray(query, f32).T.reshape(2, 128, C).transpose(1, 0, 2)
        ).astype(bf),
        "Wv": np.ascontiguousarray(
            np.asarray(W_v, f32)[0].reshape(8, 128).T).astype(bf),
        "bh": np.asarray(b_hidden, f32).reshape(1, H).astype(bf),
        "IndA": np.tile(np.eye(C, dtype=f32), (1, 8)).astype(bf),
        "IndB": np.kron(np.eye(n_pad, dtype=f32),
                        np.ones((1, C), f32)).astype(bf),
        "WlT": np.ascontiguousarray(
            np.asarray(W_lin, f32).T.reshape(8, 128, E).transpose(1, 0, 2)
        ).astype(bf),
        "bl": np.ascontiguousarray(
            np.asarray(b_lin, f32).reshape(2, 128).T).astype(f32),
        "cb": np.concatenate([np.asarray(x, f32) for x in
                              (conv_b0, conv_b1, conv_b2)]).reshape(1, -1)
        .astype(bf),
        "WcT": np.ascontiguousarray(
            np.asarray(W_cnn, f32).T.reshape(3, 128, TYPE_NUM)
            .transpose(1, 0, 2)).astype(bf),
        "bc": np.asarray(b_cnn, f32).reshape(TYPE_NUM, 1).astype(f32),
    }
    for i, w in enumerate((conv_w0, conv_w1, conv_w2)):
        w = np.asarray(w, f32)  # [NF, E, ki]
        arr = w.transpose(1, 2, 0).reshape(2, 128, KS[i], NF) \
            .transpose(1, 2, 0, 3)  # [128, ki, 2, NF]
        shared[f"cw{i}"] = np.ascontiguousarray(arr).astype(bf)

    context = np.asarray(context, f32)
    per_core = []
    for b in range(B):
        ctx_act = context[b][idxs[b]]  # [n_pad, E]
        ctx_act = ctx_act * (mads[b] == 0.0)[:, None]  # zero padded rows
        ctxT = np.ascontiguousarray(
            ctx_act.T.reshape(2, 128, n_pad).transpose(1, 0, 2))
        per_core.append({
            "ctx": np.ascontiguousarray(ctx_act).astype(bf),
            "ctxT": ctxT.astype(bf),
            "maskadd": np.tile(mads[b][None, :], (C, 1)).astype(f32),
            **shared,
        })
    return n_pad, per_core


def kernel(**inputs):
    global LAST_EXEC_NS
    n_pad, per_core = _prep_inputs(**inputs)
    key = (n_pad, os.environ.get("KSTAGE", "99"))
    if key not in _CACHE:
        _CACHE[key] = _build_program(n_pad)
    nc = _CACHE[key]
    res = run_bass_kernel_spmd(nc, per_core, list(range(NUM_CORES)),
                               trace=TRACE)
    LAST_EXEC_NS = res.exec_time_ns
    out = np.stack([res.results[i]["out"] for i in range(NUM_CORES)])
    return out.astype(np.float32)



# revision 12
# speedup vs baseline: 1.1378x; 1.1378x over previous
"""Trainium2 Bass kernel for nn_CNNPredictor (attention scorer + CNN head).

Sharding: data-parallel over batch b (8 batches -> 8 NeuronCores), no
collectives. Each core computes its batch's [TYPE_NUM] output row; host
gathers to [B, TYPE_NUM].

Math (per batch):
  pre[c,t,:] = [q|ctx| , |q-ctx|, q*ctx] @ W_h.T + b_h   (4e = 1024 hidden)
split as
  pre = A[c] + B[t] + W3 @ |q-ctx| + W4 @ (q*ctx)
with A = q @ W1.T, B = ctx @ W2.T + b_h.

Phase 1 is PAIR-MAJOR: out[pair, h] with pair=(t,c) on partitions and the
hidden dim streaming as the moving operand. Per 128-pair tile and 512-h
half: 4 main matmuls (ft chunks stationary, K=128 each) plus ONE combined
indicator matmul (K=80: rows 0:64 broadcast A over t via the c-onehot,
rows 64:80 broadcast this rt's 8 B rows, double-buffered by rt parity).
scores = Wv . tanh(pre) is a fused DVE tensor_tensor_reduce over the free
(h) axis -- no PE time and scores land per-partition, so the masked
softmax is partition-parallel with no DRAM round trip. Only t-positions
with mask==1 are computed (padded to a multiple of 8).
"""

import os
import sys

for _p in ("/opt/trn_rl_repo",):
    if _p not in sys.path:
        sys.path.append(_p)

import numpy as np
from ml_dtypes import bfloat16

import concourse.bass as bass
import concourse.bacc as bacc
import concourse.tile as tile
from concourse import mybir
from concourse.bass_utils import run_bass_kernel_spmd
from concourse.bass_interp import get_hw_module

F32 = mybir.dt.float32
BF16 = mybir.dt.bfloat16
AF = mybir.ActivationFunctionType
ALU = mybir.AluOpType

B, C, T, E = 8, 64, 128, 256
H = 4 * E  # 1024
NF, TYPE_NUM = 128, 40
KS = (5, 4, 3)
NEG = -1e10
NUM_CORES = 8

# module-level knobs for test harness
TRACE = False
LAST_EXEC_NS = None

_CACHE = {}


def _build_program(n_pad):
    """Build the SPMD Bass program for padded active length n_pad (mult of 8)."""
    stage = int(os.environ.get("KSTAGE", "99"))
    fast = set(x for x in os.environ.get("KFAST", "").split(",") if x)
    R = n_pad // 8
    R4 = 4 * R  # t-block count (2 t per block)

    nc = bacc.Bacc("TRN2", target_bir_lowering=False, debug=False,
                   num_devices=NUM_CORES)

    d_WhT = nc.dram_tensor("WhT", [128, 8, H], BF16, kind="ExternalInput")
    d_qT = nc.dram_tensor("qT", [128, 2, C], BF16, kind="ExternalInput")
    d_ctxT = nc.dram_tensor("ctxT", [128, 2, n_pad], BF16, kind="ExternalInput")
    d_ctx = nc.dram_tensor("ctx", [n_pad, E], BF16, kind="ExternalInput")
    d_WvB = nc.dram_tensor("WvB", [128, H], BF16, kind="ExternalInput")
    d_bh = nc.dram_tensor("bh", [1, H], BF16, kind="ExternalInput")
    d_maskP = nc.dram_tensor("maskP", [128, R4], F32, kind="ExternalInput")
    d_Ind2 = nc.dram_tensor("Ind2", [80, 2, 512], BF16, kind="ExternalInput")
    d_eye = nc.dram_tensor("eye", [128, 128], BF16, kind="ExternalInput")
    d_eyeF = nc.dram_tensor("eyeF", [128, 128], F32, kind="ExternalInput")
    d_WlT = nc.dram_tensor("WlT", [128, 8, E], BF16, kind="ExternalInput")
    d_bl = nc.dram_tensor("bl", [128, 2], F32, kind="ExternalInput")
    d_cw = [nc.dram_tensor(f"cw{i}", [128, KS[i], 2, NF], BF16,
                           kind="ExternalInput") for i in range(3)]
    d_cb = nc.dram_tensor("cb", [1, 3 * NF], BF16, kind="ExternalInput")
    d_WcT = nc.dram_tensor("WcT", [128, 3, TYPE_NUM], BF16, kind="ExternalInput")
    d_bc = nc.dram_tensor("bc", [TYPE_NUM, 1], F32, kind="ExternalInput")
    d_out = nc.dram_tensor("out", [TYPE_NUM], F32, kind="ExternalOutput")

    with tile.TileContext(nc) as tc:
        with (
            tc.tile_pool(name="const", bufs=1) as cpool,
            tc.tile_pool(name="ft", bufs=2) as ftpool,
            tc.tile_pool(name="th", bufs=6) as thpool,
            tc.tile_pool(name="soft", bufs=1) as spool,
            tc.tile_pool(name="ps_main", bufs=6, space="PSUM") as ps_main,
            tc.tile_pool(name="ps_sm", bufs=2, space="PSUM") as ps_sm,
        ):
            # ---- PE warmup: get HAM to K=8/8 before real matmuls ----------
            wsb = cpool.tile([128, 512], BF16)
            nc.vector.memset(wsb[:], 0.0)
            for _w in range(10):
                wps = ps_sm.tile([128, 512], F32, tag="sm")
                nc.tensor.matmul(wps[:], wsb[:, 0:128], wsb[:],
                                 start=True, stop=True)

            # ---- load constants (sync queue: critical path order) ---------
            qT = cpool.tile([128, 2, C], BF16)
            nc.sync.dma_start(out=qT[:], in_=d_qT[:])
            ctxT = cpool.tile([128, 2, n_pad], BF16)
            nc.sync.dma_start(out=ctxT[:], in_=d_ctxT[:])
            bh = cpool.tile([1, H], BF16)
            nc.sync.dma_start(out=bh[:], in_=d_bh[:])
            WhT = cpool.tile([128, 8, H], BF16)
            for kc in (4, 5, 6, 7, 0, 1, 2, 3):
                nc.sync.dma_start(out=WhT[:, kc, :], in_=d_WhT[:, kc, :])
            Ind2 = cpool.tile([80, 2, 512], BF16)
            nc.sync.dma_start(out=Ind2[:], in_=d_Ind2[:])
            WvB = cpool.tile([128, H], BF16)
            nc.sync.dma_start(out=WvB[:], in_=d_WvB[:])
            maskP = cpool.tile([128, R4], F32)
            nc.sync.dma_start(out=maskP[:], in_=d_maskP[:])
            eye = cpool.tile([128, 128], BF16)
            nc.sync.dma_start(out=eye[:], in_=d_eye[:])
            eyeF = cpool.tile([128, 128], F32)
            nc.sync.dma_start(out=eyeF[:], in_=d_eyeF[:])
            # non-critical consts on the gpsimd queue
            ctxa = cpool.tile([n_pad, E], BF16)
            nc.gpsimd.dma_start(out=ctxa[:], in_=d_ctx[:])
            WlT = cpool.tile([128, 8, E], BF16)
            nc.gpsimd.dma_start(out=WlT[:], in_=d_WlT[:])
            bl = cpool.tile([128, 2], F32)
            nc.gpsimd.dma_start(out=bl[:], in_=d_bl[:])
            cw = []
            for i in range(3):
                cwt = cpool.tile([128, KS[i], 2, NF], BF16, tag=f"cw{i}")
                nc.gpsimd.dma_start(out=cwt[:], in_=d_cw[i][:])
                cw.append(cwt)
            cb = cpool.tile([1, 3 * NF], BF16)
            nc.gpsimd.dma_start(out=cb[:], in_=d_cb[:])
            WcT = cpool.tile([128, 3, TYPE_NUM], BF16)
            nc.gpsimd.dma_start(out=WcT[:], in_=d_WcT[:])
            bc = cpool.tile([TYPE_NUM, 1], F32)
            nc.gpsimd.dma_start(out=bc[:], in_=d_bc[:])

            ones = cpool.tile([1, max(n_pad, C)], BF16)
            nc.vector.memset(ones[:], 1.0)

            # dense broadcast materializations (step-0 read APs mis-execute
            # on HW DVE): qbc[p, ec, t, c] = qT[p, ec, c]; ctxbc[p, ec, t, c]
            # = ctxT[p, ec, t] -- built by doubling copies.
            qbc = cpool.tile([128, 2, 8, C], BF16)
            nc.vector.tensor_copy(qbc[:, :, 0, :], qT[:])
            nc.vector.tensor_copy(qbc[:, :, 1, :], qbc[:, :, 0, :])
            nc.vector.tensor_copy(qbc[:, :, 2:4, :], qbc[:, :, 0:2, :])
            nc.vector.tensor_copy(qbc[:, :, 4:8, :], qbc[:, :, 0:4, :])
            ctxbc = cpool.tile([128, 2, n_pad, C], BF16)
            nc.vector.tensor_copy(ctxbc[:, :, :, 0], ctxT[:])
            w = 1
            while w < C:
                nc.vector.tensor_copy(ctxbc[:, :, :, w:2 * w],
                                      ctxbc[:, :, :, 0:w])
                w *= 2

            # ---- phase 0: AB[0:64] = q @ W1.T ; B_T = ctx @ W2.T + b_h ----
            # AB rows 64:72 / 72:80 are this/next rt's B rows (parity slots).
            AB = cpool.tile([80, H], BF16)
            nc.vector.memset(AB[:], 0.0)
            B_T = cpool.tile([n_pad, H], BF16)
            for jn in range(2):
                jsl = slice(jn * 512, (jn + 1) * 512)
                psA = ps_sm.tile([C, 512], F32, tag="sm")
                nc.tensor.matmul(psA[:], qT[:, 0, :], WhT[:, 0, jsl],
                                 start=True, stop=False)
                nc.tensor.matmul(psA[:], qT[:, 1, :], WhT[:, 1, jsl],
                                 start=False, stop=True)
                nc.scalar.copy(AB[0:64, jsl], psA[:])
                psB = ps_sm.tile([n_pad, 512], F32, tag="sm")
                nc.tensor.matmul(psB[:], ctxT[:, 0, :], WhT[:, 2, jsl],
                                 start=True, stop=False)
                nc.tensor.matmul(psB[:], ctxT[:, 1, :], WhT[:, 3, jsl],
                                 start=False, stop=False)
                nc.tensor.matmul(psB[:], ones[:, :n_pad], bh[:, jsl],
                                 start=False, stop=True)
                nc.scalar.copy(B_T[:, jsl], psB[:])

            if stage < 2:
                nc.gpsimd.dma_start(out=d_out[:], in_=AB[0:TYPE_NUM, 0])

            # ---- phase 1: pair-major scores --------------------------------
            # scoresP[(t2,c), tb] with t = tb*2 + t2, tb = rt*4 + ti
            scoresP = spool.tile([128, R4], F32)
            junk = spool.tile([128, 512], BF16)
            if stage >= 2:
                for rt in range(R):
                    par = rt % 2
                    # this rt's 8 B rows into the parity slot of AB
                    # (cross-partition move -> local DMA, not a DVE op)
                    dma_eng = nc.scalar if "scdma" in fast else nc.sync
                    dma_eng.dma_start(
                        out=AB[64 + 8 * par:72 + 8 * par, :],
                        in_=B_T[rt * 8:(rt + 1) * 8, :])
                    # ft build: ftC = |q - ctx|, ftD = q * ctx  [128,2,8,C]
                    ftC = ftpool.tile([128, 2, 8, C], BF16, tag="ftC")
                    ftD = ftpool.tile([128, 2, 8, C], BF16, tag="ftD")
                    for ec in range(2):
                        bq = qbc[:, ec]
                        bcx = ctxbc[:, ec, rt * 8:(rt + 1) * 8, :]
                        nc.vector.tensor_sub(ftC[:, ec], bq, bcx)
                        nc.vector.scalar_tensor_tensor(
                            ftC[:, ec], ftC[:, ec], -1.0, ftC[:, ec],
                            op0=ALU.mult, op1=ALU.max)
                        nc.vector.tensor_mul(ftD[:, ec], bq, bcx)
                    for ti in range(4):
                        tsl = slice(2 * ti, 2 * ti + 2)
                        lhss = [ftC[:, 0, tsl, :], ftC[:, 1, tsl, :],
                                ftD[:, 0, tsl, :], ftD[:, 1, tsl, :]]
                        lhss = [x.rearrange("p a b -> p (a b)") for x in lhss]
                        isl = Ind2[:, par, ti * 128:(ti + 1) * 128]
                        sc0 = None
                        for hf in range(2):
                            hsl = slice(hf * 512, (hf + 1) * 512)
                            P = ps_main.tile([128, 512], F32, tag="P")
                            for mi in range(4):
                                nc.tensor.matmul(P[:], lhss[mi],
                                                 WhT[:, 4 + mi, hsl],
                                                 start=(mi == 0), stop=False)
                            nc.tensor.matmul(P[:], isl, AB[:, hsl],
                                             start=False, stop=True)
                            TH = thpool.tile([128, 512], BF16, tag="TH")
                            nc.scalar.activation(TH[:], P[:], AF.Tanh)
                            scol = scoresP[:, rt * 4 + ti:rt * 4 + ti + 1]
                            if "ttr" in fast:
                                if hf == 0:
                                    sc0 = thpool.tile([128, 1], F32,
                                                      tag="sc0")
                                    nc.vector.tensor_tensor_reduce(
                                        out=junk[:], in0=TH[:],
                                        in1=WvB[:, hsl], scale=1.0,
                                        scalar=0.0,
                                        op0=ALU.mult, op1=ALU.add,
                                        accum_out=sc0[:])
                                else:
                                    nc.vector.tensor_tensor_reduce(
                                        out=junk[:], in0=TH[:],
                                        in1=WvB[:, hsl], scale=1.0,
                                        scalar=sc0[:],
                                        op0=ALU.mult, op1=ALU.add,
                                        accum_out=scol)
                            else:
                                nc.vector.tensor_mul(junk[:], TH[:],
                                                     WvB[:, hsl])
                                if hf == 0:
                                    sc0 = thpool.tile([128, 1], F32,
                                                      tag="sc0")
                                    nc.vector.tensor_reduce(
                                        sc0[:], junk[:],
                                        axis=mybir.AxisListType.X,
                                        op=ALU.add)
                                else:
                                    sc1 = thpool.tile([128, 1], F32,
                                                      tag="sc1")
                                    nc.vector.tensor_reduce(
                                        sc1[:], junk[:],
                                        axis=mybir.AxisListType.X,
                                        op=ALU.add)
                                    nc.vector.tensor_add(scol, sc0[:],
                                                         sc1[:])
            if stage == 2:
                nc.sync.dma_start(out=d_out[:], in_=scoresP[0:TYPE_NUM, 0])

            # ---- masked softmax + g = attn @ ctx ---------------------------
            # scoresP[(t2,c), tb] -> scT[c, t] via 3 PE transposes, then the
            # softmax runs partition-parallel over c with t in the free dim.
            if stage >= 3:
                nc.vector.tensor_add(scoresP[:], scoresP[:], maskP[:])
                t1_ps = ps_sm.tile([R4, 128], F32, tag="sm")
                nc.tensor.transpose(t1_ps[:], scoresP[:], eyeF[:])
                s2 = spool.tile([R4, 2, 64], F32)
                nc.vector.tensor_copy(s2[:], t1_ps[:].rearrange(
                    "p (a b) -> p a b", a=2))
                scT = spool.tile([64, R4, 2], F32)
                for t2 in range(2):
                    te_ps = ps_sm.tile([64, R4], F32, tag="sm")
                    nc.tensor.transpose(te_ps[:], s2[:, t2, :],
                                        eyeF[0:R4, 0:R4])
                    nc.vector.tensor_copy(scT[:, :, t2], te_ps[:])
                scTf = scT[:].rearrange("p a b -> p (a b)")  # [c, n_pad]
                mxp = spool.tile([64, 1], F32)
                nc.vector.tensor_reduce(mxp[:], scTf,
                                        axis=mybir.AxisListType.X, op=ALU.max)
                mx = spool.tile([64, 1], F32)
                nc.vector.tensor_scalar_mul(mx[:], mxp[:], -1.0)
                ex = spool.tile([64, n_pad], F32)
                se = spool.tile([64, 1], F32)
                nc.scalar.activation(ex[:], scTf, AF.Exp, bias=mx[:],
                                     scale=1.0, accum_out=se[:])
                rse = spool.tile([64, 1], F32)
                nc.vector.reciprocal(rse[:], se[:])
                attn = spool.tile([64, n_pad], BF16)
                nc.vector.tensor_scalar_mul(attn[:], ex[:], rse[:])
                attnT_ps = ps_sm.tile([n_pad, C], BF16, tag="sm")
                nc.tensor.transpose(attnT_ps[:], attn[:], eye[0:64, 0:64])
                attnT = spool.tile([n_pad, C], BF16)
                nc.vector.tensor_copy(attnT[:], attnT_ps[:])
                g_ps = ps_sm.tile([C, E], F32, tag="sm")
                nc.tensor.matmul(g_ps[:], attnT[:], ctxa[:], start=True,
                                 stop=True)
                g_sb = spool.tile([C, E], BF16)
                nc.scalar.copy(g_sb[:], g_ps[:])
                gT = spool.tile([128, 2, C], BF16)
                for ec in range(2):
                    gT_ps = ps_sm.tile([128, C], BF16, tag="sm")
                    nc.tensor.transpose(gT_ps[:],
                                        g_sb[:, ec * 128:(ec + 1) * 128],
                                        eye[0:64, 0:64])
                    nc.vector.tensor_copy(gT[:, ec, :], gT_ps[:])
            if stage == 3:
                nc.sync.dma_start(out=d_out[:], in_=g_sb[0:TYPE_NUM, 0])

            # ---- phase 2: h2 = tanh([q|g|,|q-g|,q*g] @ Wh.T + bh) ----------
            if stage >= 4:
                f2C = spool.tile([128, 2, C], BF16)
                f2D = spool.tile([128, 2, C], BF16)
                for ec in range(2):
                    nc.vector.tensor_sub(f2C[:, ec], qT[:, ec, :], gT[:, ec, :])
                    nc.vector.scalar_tensor_tensor(
                        f2C[:, ec], f2C[:, ec], -1.0, f2C[:, ec],
                        op0=ALU.mult, op1=ALU.max)
                    nc.vector.tensor_mul(f2D[:, ec], qT[:, ec, :], gT[:, ec, :])
                h2T = spool.tile([128, 8, C], BF16)
                for jc in range(8):
                    jsl = slice(jc * 128, (jc + 1) * 128)
                    H2 = ps_sm.tile([128, C], F32, tag="sm")
                    for mi, rhs_t in enumerate((qT[:, 0, :], qT[:, 1, :],
                                                gT[:, 0, :], gT[:, 1, :],
                                                f2C[:, 0, :], f2C[:, 1, :],
                                                f2D[:, 0, :], f2D[:, 1, :])):
                        nc.tensor.matmul(H2[:], WhT[:, mi, jsl], rhs_t,
                                         start=(mi == 0), stop=False)
                    nc.tensor.matmul(H2[:], bh[:, jsl], ones[:, :C],
                                     start=False, stop=True)
                    nc.scalar.activation(h2T[:, jc, :], H2[:], AF.Tanh)

                # x.T = W_lin @ h2 : [e, c], e-major for the convs
                xT = spool.tile([128, 2, C], BF16)
                for ec2 in range(2):
                    X = ps_sm.tile([128, C], F32, tag="sm")
                    for jc in range(8):
                        nc.tensor.matmul(
                            X[:], WlT[:, jc, ec2 * 128:(ec2 + 1) * 128],
                            h2T[:, jc, :], start=(jc == 0), stop=(jc == 7))
                    nc.scalar.activation(xT[:, ec2, :], X[:], AF.Identity,
                                         bias=bl[:, ec2:ec2 + 1], scale=1.0)

                # convs + relu + maxpool; pooled[f, i]
                pooled_raw = spool.tile([NF, 3], F32)
                for i in range(3):
                    ki = KS[i]
                    oi = C - ki + 1
                    Y = ps_sm.tile([NF, oi], F32, tag="sm")
                    first = True
                    for dk in range(ki):
                        for ec2 in range(2):
                            nc.tensor.matmul(Y[:], cw[i][:, dk, ec2, :],
                                             xT[:, ec2, dk:dk + oi],
                                             start=first, stop=False)
                            first = False
                    nc.tensor.matmul(Y[:], cb[:, i * NF:(i + 1) * NF],
                                     ones[:, :oi], start=False, stop=True)
                    nc.vector.tensor_reduce(pooled_raw[:, i:i + 1], Y[:],
                                            axis=mybir.AxisListType.X,
                                            op=ALU.max)
                pooled = spool.tile([NF, 3], BF16)
                nc.scalar.activation(pooled[:], pooled_raw[:], AF.Relu)

                # final linear: out = W_cnn @ cnn + b_cnn
                O = ps_sm.tile([TYPE_NUM, 1], F32, tag="sm")
                for i in range(3):
                    nc.tensor.matmul(O[:], WcT[:, i, :], pooled[:, i:i + 1],
                                     start=(i == 0), stop=(i == 2))
                out_sb = spool.tile([TYPE_NUM, 1], F32)
                nc.scalar.activation(out_sb[:], O[:], AF.Identity, bias=bc[:],
                                     scale=1.0)
                nc.sync.dma_start(out=d_out[:], in_=out_sb[:, 0])

    nc.compile()
    nc.m = get_hw_module(nc.m)
    return nc


def _prep_inputs(query, context, mask, W_hidden, b_hidden, W_v, b_v,
                 W_lin, b_lin, conv_w0, conv_b0, conv_w1, conv_b1,
                 conv_w2, conv_b2, W_cnn, b_cnn):
    """Host-side layout prep. Returns (n_pad, per_core_maps)."""
    f32 = np.float32
    mask = np.asarray(mask)
    n_act = mask.sum(1)
    if n_act.min() == 0:
        # degenerate: keep every position, mask on device via maskP
        idxs = [np.arange(T) for _ in range(B)]
        n_pad = T
        valids = [(mask[b] >= 1) for b in range(B)]
    else:
        n_pad = max(8, int(-(-int(n_act.max()) // 8) * 8))
        idxs, valids = [], []
        for b in range(B):
            idx = np.nonzero(mask[b])[0]
            v = np.zeros(n_pad, bool)
            v[:len(idx)] = True
            idx = np.concatenate([idx, np.zeros(n_pad - len(idx), np.int64)])
            idxs.append(idx)
            valids.append(v)
    R = n_pad // 8
    R4 = 4 * R

    bf = bfloat16
    Wh = np.asarray(W_hidden, f32)
    WhT = np.ascontiguousarray(Wh.T).reshape(8, 128, H).transpose(1, 0, 2)

    # combined indicator [80, 2(parity), 512]: col p of tile ti encodes the
    # pair (t2, c) with t2 = (p % 128) // 64, c = p % 64, tb-slot j = 2*ti+t2
    Ind2 = np.zeros((80, 2, 512), f32)
    for p in range(512):
        ti = p // 128
        t2 = (p % 128) // 64
        c = p % 64
        j = 2 * ti + t2
        Ind2[c, 0, p] = 1.0
        Ind2[c, 1, p] = 1.0
        Ind2[64 + j, 0, p] = 1.0
        Ind2[72 + j, 1, p] = 1.0

    shared = {
        "WhT": np.ascontiguousarray(WhT).astype(bf),
        "qT": np.ascontiguousarray(
            np.asarray(query, f32).T.reshape(2, 128, C).transpose(1, 0, 2)
        ).astype(bf),
        "WvB": np.ascontiguousarray(
            np.tile(np.asarray(W_v, f32).reshape(1, H), (128, 1))).astype(bf),
        "bh": np.asarray(b_hidden, f32).reshape(1, H).astype(bf),
        "Ind2": Ind2.astype(bf),
        "eye": np.eye(128, dtype=f32).astype(bf),
        "eyeF": np.eye(128, dtype=f32),
        "WlT": np.ascontiguousarray(
            np.asarray(W_lin, f32).T.reshape(8, 128, E).transpose(1, 0, 2)
        ).astype(bf),
        "bl": np.ascontiguousarray(
            np.asarray(b_lin, f32).reshape(2, 128).T).astype(f32),
        "cb": np.concatenate([np.asarray(x, f32) for x in
                              (conv_b0, conv_b1, conv_b2)]).reshape(1, -1)
        .astype(bf),
        "WcT": np.ascontiguousarray(
            np.asarray(W_cnn, f32).T.reshape(3, 128, TYPE_NUM)
            .transpose(1, 0, 2)).astype(bf),
        "bc": np.asarray(b_cnn, f32).reshape(TYPE_NUM, 1).astype(f32),
    }
    for i, w in enumerate((conv_w0, conv_w1, conv_w2)):
        w = np.asarray(w, f32)  # [NF, E, ki]
        arr = w.transpose(1, 2, 0).reshape(2, 128, KS[i], NF) \
            .transpose(1, 2, 0, 3)  # [128, ki, 2, NF]
        shared[f"cw{i}"] = np.ascontiguousarray(arr).astype(bf)

    context = np.asarray(context, f32)
    per_core = []
    for b in range(B):
        ctx_act = context[b][idxs[b]]  # [n_pad, E]
        ctx_act = ctx_act * valids[b][:, None]  # zero padded rows
        ctxT = np.ascontiguousarray(
            ctx_act.T.reshape(2, 128, n_pad).transpose(1, 0, 2))
        # maskP[p, tb]: t = tb*2 + p//64 ; NEG where padded/invalid
        maskP = np.zeros((128, R4), f32)
        for p in range(128):
            t2 = p // 64
            tt = np.arange(R4) * 2 + t2
            maskP[p, :] = np.where(valids[b][tt], 0.0, NEG)
        per_core.append({
            "ctx": np.ascontiguousarray(ctx_act).astype(bf),
            "ctxT": ctxT.astype(bf),
            "maskP": maskP,
            **shared,
        })
    return n_pad, per_core


def kernel(**inputs):
    global LAST_EXEC_NS
    n_pad, per_core = _prep_inputs(**inputs)
    key = (n_pad, os.environ.get("KSTAGE", "99"))
    if key not in _CACHE:
        _CACHE[key] = _build_program(n_pad)
    nc = _CACHE[key]
    res = run_bass_kernel_spmd(nc, per_core, list(range(NUM_CORES)),
                               trace=TRACE)
    LAST_EXEC_NS = res.exec_time_ns
    out = np.stack([res.results[i]["out"] for i in range(NUM_CORES)])
    return out.astype(np.float32)
